# revision 4
# baseline (speedup 1.0000x reference)
"""ConvLSTM2D cell on 8 Trainium2 NeuronCores.

Data-parallel over batch: B=16 -> 2 images per core. The 3x3 conv is
computed with a rank-5 Karatsuba factorization of the width dimension:
for each output pair (2t, 2t+1) the 3-tap conv along W decomposes into
5 products
    P = w0*x0 + (w2-w1)*x2        y0 = P + T
    T = w1*(x1 + x2)              y1 = T + S
    S = (w0-w1)*x1 + w2*x3
(x_j = input col 2t-1+j). Each product contracts channels and the 3 kh
taps on the tensor engine and accumulates into its own PSUM bank, so a
gate needs only 23 N=512 matmul slots per 1024 pixels vs 28 for direct
conv (1.22x fewer PE columns, the kernel is PE-bound). The combine
y0=P+T / y1=T+S costs just 2 VectorE adds per 1024 px. Host precomputes
the parity planes (even cols, odd cols, pairwise sums) and the shifted
duplicate-channel packings that let the 64-channel x taps pair up into
K=128 matmuls. ScalarE applies bias+sigmoid/tanh; VectorE does the LSTM
elementwise math in fp16 (2x mode); outputs stored as parity planes and
re-interleaved on host.
"""

import sys

if "/opt/trn_rl_repo" not in sys.path:
    sys.path.insert(0, "/opt/trn_rl_repo")

import numpy as np

import concourse.bass as bass
import concourse.tile as tile
from concourse import bacc, mybir
from concourse.bass_utils import run_bass_kernel_spmd

N_CORES = 8
B, C_IN, C_HID, H, W = 16, 64, 128, 64, 64
B_LOC = B // N_CORES  # 2 images per core
HP = H + 2  # padded rows (input rows -1..64)
TW = W // 2 + 1  # 33 tile columns (incl. one pad col)
NT = W // 2  # 32 tiles used per row
QROWS = 16  # output rows per quarter -> N=512 per matmul
NQ = H // QROWS  # 4 quarters per image
# instance order: P1, P2, T, S1, S2
INST = 5

_cache = {}


def _build(dt_mm=mybir.dt.float16, trace=False, unroll=1):
    key = (dt_mm, trace, unroll)
    if key in _cache:
        return _cache[key]
    f32 = mybir.dt.float32
    dt_ep = dt_mm if mybir.dt.size(dt_mm) == 2 else f32  # epilogue dtype
    nc = bacc.Bacc("TRN2", target_bir_lowering=False, debug=False, num_devices=N_CORES)

    # h parity planes: hO = odd' cols (-1,1,..,63), hE = even cols (0,2,..,64),
    # hS = pairwise sums col(2t)+col(2t+1). All [C_HID, HP, TW].
    hE_ap = nc.dram_tensor("hE", [B_LOC, C_HID, HP, TW], dt_mm, kind="ExternalInput").ap()
    hO_ap = nc.dram_tensor("hO", [B_LOC, C_HID, HP, TW], dt_mm, kind="ExternalInput").ap()
    hS_ap = nc.dram_tensor("hS", [B_LOC, C_HID, HP, TW], dt_mm, kind="ExternalInput").ap()
    # x dup planes [2*C_IN, HP, TW]: ch 0-63 = plane, ch 64-127 = plane shifted
    # one row down (kh0+kh1 pairing) or one tile col (kh2 instance pairing).
    xP01_ap = nc.dram_tensor("xP01", [B_LOC, 2 * C_IN, HP, TW], dt_mm, kind="ExternalInput").ap()
    xS01_ap = nc.dram_tensor("xS01", [B_LOC, 2 * C_IN, HP, TW], dt_mm, kind="ExternalInput").ap()
    xT01_ap = nc.dram_tensor("xT01", [B_LOC, 2 * C_IN, HP, TW], dt_mm, kind="ExternalInput").ap()
    xPk2_ap = nc.dram_tensor("xPk2", [B_LOC, 2 * C_IN, HP, TW], dt_mm, kind="ExternalInput").ap()
    xSk2_ap = nc.dram_tensor("xSk2", [B_LOC, 2 * C_IN, HP, TW], dt_mm, kind="ExternalInput").ap()
    # c parity planes [C_HID, H*NT]
    cE_ap = nc.dram_tensor("cE", [B_LOC, C_HID, H * NT], dt_ep, kind="ExternalInput").ap()
    cO_ap = nc.dram_tensor("cO", [B_LOC, C_HID, H * NT], dt_ep, kind="ExternalInput").ap()
    # weights: wH [C_HID, 5 inst * 3 kh, 4*C_HID]; wX01 [2*C_IN, 5 inst, 4*C_HID]
    # (kh0;kh1 stacked); wXk2 [2*C_IN, 2 pair, 4*C_HID] (P1;P2 and S1;S2 at kh2);
    # wXT2 [C_IN, 4*C_HID] (lone T kh2)
    wH_ap = nc.dram_tensor("wH", [INST * 3, C_HID, 4 * C_HID], dt_mm, kind="ExternalInput").ap()
    wX01_ap = nc.dram_tensor("wX01", [INST, 2 * C_IN, 4 * C_HID], dt_mm, kind="ExternalInput").ap()
    wXk2_ap = nc.dram_tensor("wXk2", [2, 2 * C_IN, 4 * C_HID], dt_mm, kind="ExternalInput").ap()
    wXT2_ap = nc.dram_tensor("wXT2", [C_IN, 4 * C_HID], dt_mm, kind="ExternalInput").ap()
    bias_ap = nc.dram_tensor("biasT", [C_HID, 4], f32, kind="ExternalInput").ap()
    # outputs as parity planes
    hnE_ap = nc.dram_tensor("hnE", [B_LOC, C_HID, H * NT], dt_ep, kind="ExternalOutput").ap()
    hnO_ap = nc.dram_tensor("hnO", [B_LOC, C_HID, H * NT], dt_ep, kind="ExternalOutput").ap()
    cnE_ap = nc.dram_tensor("cnE", [B_LOC, C_HID, H * NT], dt_ep, kind="ExternalOutput").ap()
    cnO_ap = nc.dram_tensor("cnO", [B_LOC, C_HID, H * NT], dt_ep, kind="ExternalOutput").ap()

    with tile.TileContext(nc) as tc:
        with (
            tc.tile_pool(name="weights", bufs=1) as wpool,
            tc.tile_pool(name="imgs", bufs=2) as ipool,
            tc.tile_pool(name="cstate", bufs=3) as cpool,
            tc.tile_pool(name="psum", bufs=2, space="PSUM") as ppool,
            tc.tile_pool(name="acts", bufs=2) as apool,
            tc.tile_pool(name="outs", bufs=3) as opool,
        ):
            wH_t = wpool.tile([C_HID, INST * 3, 4 * C_HID], dt_mm, tag="wH")
            wX01_t = wpool.tile([2 * C_IN, INST, 4 * C_HID], dt_mm, tag="wX01")
            wXk2_t = wpool.tile([2 * C_IN, 2, 4 * C_HID], dt_mm, tag="wXk2")
            wXT2_t = wpool.tile([C_IN, 4 * C_HID], dt_mm, tag="wXT2")
            bias_t = wpool.tile([C_HID, 4], f32, tag="bias")
            nc.sync.dma_start(wH_t[:], wH_ap.rearrange("t k m -> k t m"))
            nc.sync.dma_start(wX01_t[:], wX01_ap.rearrange("t k m -> k t m"))
            nc.sync.dma_start(wXk2_t[:], wXk2_ap.rearrange("t k m -> k t m"))
            nc.sync.dma_start(wXT2_t[:], wXT2_ap[:])
            nc.sync.dma_start(bias_t[:], bias_ap[:])

            planes = []
            for b in range(B_LOC):
                pl = {}
                for name, ap_, npart in (
                    ("hE", hE_ap, C_HID),
                    ("hO", hO_ap, C_HID),
                    ("hS", hS_ap, C_HID),
                    ("xP01", xP01_ap, 2 * C_IN),
                    ("xS01", xS01_ap, 2 * C_IN),
                    ("xT01", xT01_ap, 2 * C_IN),
                    ("xPk2", xPk2_ap, 2 * C_IN),
                    ("xSk2", xSk2_ap, 2 * C_IN),
                ):
                    t = ipool.tile([npart, HP, TW], dt_mm, tag=name)
                    nc.sync.dma_start(t[:], ap_[b])
                    pl[name] = t
                planes.append(pl)

            sig = mybir.ActivationFunctionType.Sigmoid
            tanh = mybir.ActivationFunctionType.Tanh

            for _rep in range(unroll):
                for b in range(B_LOC):
                    pl = planes[b]
                    for q in range(NQ):
                        r0 = q * QROWS
                        csl = slice(r0 * NT, (r0 + QROWS) * NT)
                        gateE = []
                        gateO = []
                        for g in range(4):
                            gsl = slice(g * C_HID, (g + 1) * C_HID)
                            P = ppool.tile([C_HID, QROWS * NT], f32, tag="P")
                            T = ppool.tile([C_HID, QROWS * NT], f32, tag="T")
                            S = ppool.tile([C_HID, QROWS * NT], f32, tag="S")
                            # ---- bank P: inst P1 (@t), P2 (@t+1) ----
                            for i_inst, (hpl, c0) in enumerate((("hO", 0), ("hO", 1))):
                                inst = i_inst  # 0=P1, 1=P2
                                for kh in range(3):
                                    nc.tensor.matmul(
                                        P[:],
                                        wH_t[:, inst * 3 + kh, gsl],
                                        pl[hpl][:, r0 + kh : r0 + kh + QROWS, c0 : c0 + NT],
                                        start=(kh == 0 and i_inst == 0),
                                        stop=False,
                                    )
                                nc.tensor.matmul(
                                    P[:],
                                    wX01_t[:, inst, gsl],
                                    pl["xP01"][:, r0 : r0 + QROWS, c0 : c0 + NT],
                                    start=False,
                                    stop=False,
                                )
                            nc.tensor.matmul(
                                P[:],
                                wXk2_t[:, 0, gsl],
                                pl["xPk2"][:, r0 + 2 : r0 + 2 + QROWS, 0:NT],
                                start=False,
                                stop=True,
                            )
                            # ---- bank T: inst T (idx 2) ----
                            for kh in range(3):
                                nc.tensor.matmul(
                                    T[:],
                                    wH_t[:, 2 * 3 + kh, gsl],
                                    pl["hS"][:, r0 + kh : r0 + kh + QROWS, 0:NT],
                                    start=(kh == 0),
                                    stop=False,
                                )
                            nc.tensor.matmul(
                                T[:],
                                wX01_t[:, 2, gsl],
                                pl["xT01"][:, r0 : r0 + QROWS, 0:NT],
                                start=False,
                                stop=False,
                            )
                            nc.tensor.matmul(
                                T[:],
                                wXT2_t[:, gsl],
                                pl["xT01"][0:C_IN, r0 + 2 : r0 + 2 + QROWS, 0:NT],
                                start=False,
                                stop=True,
                            )
                            # ---- bank S: inst S1 (@t), S2 (@t+1) ----
                            for i_inst, c0 in enumerate((0, 1)):
                                inst = 3 + i_inst
                                for kh in range(3):
                                    nc.tensor.matmul(
                                        S[:],
                                        wH_t[:, inst * 3 + kh, gsl],
                                        pl["hE"][:, r0 + kh : r0 + kh + QROWS, c0 : c0 + NT],
                                        start=(kh == 0 and i_inst == 0),
                                        stop=False,
                                    )
                                nc.tensor.matmul(
                                    S[:],
                                    wX01_t[:, inst, gsl],
                                    pl["xS01"][:, r0 : r0 + QROWS, c0 : c0 + NT],
                                    start=False,
                                    stop=False,
                                )
                            nc.tensor.matmul(
                                S[:],
                                wXk2_t[:, 1, gsl],
                                pl["xSk2"][:, r0 + 2 : r0 + 2 + QROWS, 0:NT],
                                start=False,
                                stop=True,
                            )
                            # ---- combine + activation ----
                            # (TT with two PSUM operands is illegal: PSUM has
                            # one DVE read port. ScalarE stages T into SBUF.)
                            T_sb = apool.tile([C_HID, QROWS * NT], f32, tag="Tsb")
                            nc.scalar.copy(T_sb[:], T[:])
                            y0 = apool.tile([C_HID, QROWS * NT], dt_ep, tag="y0")
                            y1 = apool.tile([C_HID, QROWS * NT], dt_ep, tag="y1")
                            nc.vector.tensor_add(y0[:], P[:], T_sb[:])
                            nc.vector.tensor_add(y1[:], S[:], T_sb[:])
                            func = tanh if g == 3 else sig
                            aE = apool.tile([C_HID, QROWS * NT], dt_ep, tag=f"aE{g}")
                            aO = apool.tile([C_HID, QROWS * NT], dt_ep, tag=f"aO{g}")
                            nc.scalar.activation(aE[:], y0[:], func, bias=bias_t[:, g : g + 1])
                            nc.scalar.activation(aO[:], y1[:], func, bias=bias_t[:, g : g + 1])
                            gateE.append(aE)
                            gateO.append(aO)

                        for par, gates, c_ap, cn_ap_, hn_ap_ in (
                            (0, gateE, cE_ap, cnE_ap, hnE_ap),
                            (1, gateO, cO_ap, cnO_ap, hnO_ap),
                        ):
                            i_t, f_t, o_t, g_t = gates
                            c_sl = cpool.tile([C_HID, QROWS * NT], dt_ep, tag="c",
                                              name=f"c_{_rep}_{b}_{q}_{par}")
                            nc.sync.dma_start(c_sl[:], c_ap[b][:, csl])
                            ig = opool.tile([C_HID, QROWS * NT], dt_ep, tag="ig")
                            nc.vector.tensor_mul(ig[:], i_t[:], g_t[:])
                            fc = opool.tile([C_HID, QROWS * NT], dt_ep, tag="fc")
                            nc.vector.tensor_mul(fc[:], f_t[:], c_sl[:])
                            cn_t = opool.tile([C_HID, QROWS * NT], dt_ep, tag="cn")
                            nc.vector.tensor_add(cn_t[:], fc[:], ig[:])
                            nc.sync.dma_start(cn_ap_[b][:, csl], cn_t[:])
                            th_t = opool.tile([C_HID, QROWS * NT], dt_ep, tag="th")
                            nc.scalar.activation(th_t[:], cn_t[:], tanh)
                            hn_t = opool.tile([C_HID, QROWS * NT], dt_ep, tag="hn")
                            nc.vector.tensor_mul(hn_t[:], o_t[:], th_t[:])
                            nc.sync.dma_start(hn_ap_[b][:, csl], hn_t[:])

    nc.compile()
    _cache[key] = nc
    return nc


def _prep_inputs(x, h_cur, c_cur, weight, bias, dt_mm):
    """Host-side transform/shard. Returns in_maps for the 8 cores."""
    if dt_mm == mybir.dt.bfloat16:
        import ml_dtypes

        np_dt = ml_dtypes.bfloat16
    elif dt_mm == mybir.dt.float16:
        np_dt = np.float16
    else:
        np_dt = np.float32
    ep_dt = np_dt if np.dtype(np_dt).itemsize == 2 else np.float32
    cast = lambda a: np.ascontiguousarray(a, dtype=np_dt)

    # ---- weights: [4*C_HID, 192, 3, 3] -> [kh, kw, ci, co] ----
    wt = np.ascontiguousarray(weight.transpose(2, 3, 1, 0)).astype(np.float32)
    # rank-5 instance weights along kw: P1=w0, P2=w2-w1, T=w1, S1=w0-w1, S2=w2
    u = [
        wt[:, 0],           # P1 [kh, ci, co]
        wt[:, 2] - wt[:, 1],  # P2
        wt[:, 1],           # T
        wt[:, 0] - wt[:, 1],  # S1
        wt[:, 2],           # S2
    ]
    uh = [ui[:, C_IN:, :] for ui in u]  # [3, 128, 512] each
    ux = [ui[:, :C_IN, :] for ui in u]  # [3, 64, 512] each
    # wH: [inst*3, 128, 512]
    wH = cast(np.concatenate([uh[i] for i in range(INST)], axis=0))
    # wX01: [inst, 128, 512] = [kh0; kh1] stacked along ci
    wX01 = cast(np.stack([np.concatenate([ux[i][0], ux[i][1]], axis=0) for i in range(INST)]))
    # wXk2: [2, 128, 512] = kh2 of [P1; P2] and [S1; S2]
    wXk2 = cast(np.stack([
        np.concatenate([ux[0][2], ux[1][2]], axis=0),
        np.concatenate([ux[3][2], ux[4][2]], axis=0),
    ]))
    wXT2 = cast(ux[2][2])  # [64, 512]
    biasT = np.ascontiguousarray(bias.reshape(4, C_HID).T, dtype=np.float32)

    # ---- image parity planes ----
    def parity_planes(img, nch):
        # img: [B, nch, H, W] float32. Returns even/odd'/sum planes
        # [B, nch, HP, TW] where rows are padded (-1..H) and
        # even[t]=col 2t (t=0..32, col 64=pad), odd'[t]=col 2t-1, sum[t]=col2t+col2t+1.
        pe = np.zeros((B, nch, HP, TW), np.float32)
        po = np.zeros((B, nch, HP, TW), np.float32)
        ps = np.zeros((B, nch, HP, TW), np.float32)
        pe[:, :, 1 : H + 1, :NT] = img[:, :, :, 0::2]
        po[:, :, 1 : H + 1, 1 : NT + 1] = img[:, :, :, 1::2]
        # odd'[0] = col -1 = 0 pad; odd'[t] = col 2t-1 = odd col index t-1
        ps[:, :, 1 : H + 1, :NT] = img[:, :, :, 0::2] + img[:, :, :, 1::2]
        return pe, po, ps

    x = np.asarray(x, np.float32)
    h_cur = np.asarray(h_cur, np.float32)
    xe, xo, xs = parity_planes(x, C_IN)
    he, ho, hs = parity_planes(h_cur, C_HID)

    def rowdup(p):
        # [B, 64, HP, TW] -> [B, 128, HP, TW]; ch 64-127 = shifted one row down
        out = np.zeros((B, 2 * C_IN, HP, TW), np.float32)
        out[:, :C_IN] = p
        out[:, C_IN:, : HP - 1] = p[:, :, 1:]
        return out

    def tiledup(p):
        # ch 64-127 = shifted one tile col
        out = np.zeros((B, 2 * C_IN, HP, TW), np.float32)
        out[:, :C_IN] = p
        out[:, C_IN:, :, : TW - 1] = p[:, :, :, 1:]
        return out

    xP01 = rowdup(xo)
    xS01 = rowdup(xe)
    xT01 = rowdup(xs)
    xPk2 = tiledup(xo)
    xSk2 = tiledup(xe)

    c3 = np.asarray(c_cur, np.float32).reshape(B, C_HID, H, W)
    cE = np.ascontiguousarray(c3[:, :, :, 0::2].reshape(B, C_HID, H * NT), dtype=ep_dt)
    cO = np.ascontiguousarray(c3[:, :, :, 1::2].reshape(B, C_HID, H * NT), dtype=ep_dt)

    in_maps = []
    for i in range(N_CORES):
        s = slice(i * B_LOC, (i + 1) * B_LOC)
        in_maps.append(
            {
                "hE": cast(he[s]),
                "hO": cast(ho[s]),
                "hS": cast(hs[s]),
                "xP01": cast(xP01[s]),
                "xS01": cast(xS01[s]),
                "xT01": cast(xT01[s]),
                "xPk2": cast(xPk2[s]),
                "xSk2": cast(xSk2[s]),
                "cE": cE[s],
                "cO": cO[s],
                "wH": wH,
                "wX01": wX01,
                "wXk2": wXk2,
                "wXT2": wXT2,
                "biasT": biasT,
            }
        )
    return in_maps


def _assemble(res, key_e, key_o, ep_np):
    full_e = np.concatenate([res.results[i][key_e] for i in range(N_CORES)], axis=0)
    full_o = np.concatenate([res.results[i][key_o] for i in range(N_CORES)], axis=0)
    out = np.empty((B, C_HID, H, W), np.float32)
    out[:, :, :, 0::2] = full_e.reshape(B, C_HID, H, NT).astype(np.float32)
    out[:, :, :, 1::2] = full_o.reshape(B, C_HID, H, NT).astype(np.float32)
    return out


def run(x, h_cur, c_cur, weight, bias, dt_mm=mybir.dt.float16, trace=False):
    x = np.asarray(x)
    h_cur = np.asarray(h_cur)
    c_cur = np.asarray(c_cur)
    weight = np.asarray(weight)
    bias = np.asarray(bias)
    nc = _build(dt_mm, trace)
    in_maps = _prep_inputs(x, h_cur, c_cur, weight, bias, dt_mm)
    res = run_bass_kernel_spmd(nc, in_maps, list(range(N_CORES)), trace=trace)
    hn = _assemble(res, "hnE", "hnO", None)
    cn = _assemble(res, "cnE", "cnO", None)
    return (hn, cn), res


def kernel(x, h_cur, c_cur, weight, bias):
    (hn, cn), _ = run(x, h_cur, c_cur, weight, bias)
    return hn, cn


def _make_timing_fn(nc, in_maps):
    """Non-donating jitted runner with device-resident inputs, for
    throughput timing (slope of wall time vs iteration count)."""
    import jax
    from jax.sharding import NamedSharding

    from concourse import bass2jax, mybir as _mybir

    bass2jax.install_neuronx_cc_hook()
    n_cores = len(in_maps)
    partition_name = nc.partition_id_tensor.name if nc.partition_id_tensor else None
    in_names, out_names, out_avals, zero_outs = [], [], [], []
    for alloc in nc.m.functions[0].allocations:
        if not isinstance(alloc, _mybir.MemoryLocationSet):
            continue
        name = alloc.memorylocations[0].name
        if alloc.kind == "ExternalInput":
            if name != partition_name:
                in_names.append(name)
        elif alloc.kind == "ExternalOutput":
            out_names.append(name)
            shape = tuple(alloc.tensor_shape)
            dtype = _mybir.dt.np(alloc.dtype)
            out_avals.append(jax.core.ShapedArray(shape, dtype))
            zero_outs.append(np.zeros(shape, dtype))
    n_params = len(in_names)
    all_in_names = list(in_names) + list(out_names)
    if partition_name is not None:
        all_in_names.append(partition_name)

    def _body(*args):
        operands = list(args)
        if partition_name is not None:
            operands.append(bass2jax.partition_id_tensor())
        outs = bass2jax._bass_exec_p.bind(
            *operands,
            out_avals=tuple(out_avals),
            in_names=tuple(all_in_names),
            out_names=tuple(out_names),
            lowering_input_output_aliases=(),
            sim_require_finite=True,
            sim_require_nnan=True,
            nc=nc,
        )
        return tuple(outs)

    devices = jax.devices()[:n_cores]
    mesh = bass2jax.Mesh(np.asarray(devices), ("core",))
    in_specs = (bass2jax.PartitionSpec("core"),) * (n_params + len(out_names))
    out_specs = (bass2jax.PartitionSpec("core"),) * len(out_names)
    fn = jax.jit(
        bass2jax.shard_map(
            _body, mesh=mesh, in_specs=in_specs, out_specs=out_specs, check_rep=False
        ),
        keep_unused=True,
    )
    per_core = [[np.asarray(m[name]) for name in in_names] for m in in_maps]
    concat_in = [
        np.concatenate([per_core[c][i] for c in range(n_cores)], axis=0)
        for i in range(n_params)
    ]
    concat_zeros = [
        np.zeros((n_cores * z.shape[0], *z.shape[1:]), z.dtype) for z in zero_outs
    ]
    sh = NamedSharding(mesh, bass2jax.PartitionSpec("core"))
    dev_args = [jax.device_put(a, sh) for a in concat_in + concat_zeros]
    return fn, dev_args


def bench(x, h_cur, c_cur, weight, bias, dt_mm=None, ks=(4, 16)):
    """Returns estimated per-call device exec time in ns (pipelined slope)."""
    import time as _time

    import jax

    if dt_mm is None:
        dt_mm = mybir.dt.float16
    nc = _build(dt_mm)
    in_maps = _prep_inputs(
        np.asarray(x), np.asarray(h_cur), np.asarray(c_cur), np.asarray(weight), np.asarray(bias), dt_mm
    )
    fn, dev_args = _make_timing_fn(nc, in_maps)
    # warmup (compile + first exec)
    for _ in range(2):
        outs = fn(*dev_args)
        jax.block_until_ready(outs)

    def timed(k):
        t0 = _time.perf_counter()
        outs = None
        for _ in range(k):
            outs = fn(*dev_args)
        jax.block_until_ready(outs)
        return _time.perf_counter() - t0

    times = {}
    for k in ks:
        times[k] = min(timed(k) for _ in range(3))
    k_lo, k_hi = min(ks), max(ks)
    slope = (times[k_hi] - times[k_lo]) / (k_hi - k_lo)
    return slope * 1e9, times
